# revision 7
# baseline (speedup 1.0000x reference)
"""Trainium2 Bass kernel for causal self-attention (single head, scaled by full C).

Reference math (per batch element b, T=256, C=384):
    qkv = x @ W_qkv + b_qkv ;  q,k,v = split(qkv)
    S   = q @ k^T / sqrt(C) ; causal mask ; P = softmax(S)
    y   = (P @ v) @ W_proj + b_proj

Kernel restructuring (validated numerically to ~3.6e-3 rel err in bf16):
  * Fuse A = W_q W_k^T / sqrt(C) once per kernel; then
        S[t,s] = (x A x^T)[t,s] + f[s] + (terms constant over s -> dropped,
        softmax-invariant), with f = x @ (W_k b_q) / sqrt(C).
  * Work in the transposed score layout S'[s,t] so softmax statistics reduce
    over the partition dim via tiny matmuls (denominator = ones-matmul), and
    no max-subtraction is needed (|S| <= ~2, exp can't overflow).
  * All matmul operands bf16 (full-rate PE), all accumulation fp32 in PSUM.
  * Data-parallel over batch: B=512 -> 64 per core x 8 cores.
"""

import numpy as np

import concourse.bacc as bacc
import concourse.bass as bass
import concourse.mybir as mybir
import concourse.tile as tile
from concourse.bass import ts
from concourse.bass_utils import run_bass_kernel_spmd
from concourse.masks import make_identity, make_upper_triangular

T = 256
C = 384
N_CORES = 8
B_FULL = 512
B_PER_CORE = B_FULL // N_CORES  # 64

BF = mybir.dt.bfloat16
F32 = mybir.dt.float32
EXP = mybir.ActivationFunctionType.Exp
INV_SQRT_C = 1.0 / float(np.sqrt(C))


def build(bpc: int = B_PER_CORE) -> bass.Bass:
    nc = bacc.Bacc(
        "TRN2",
        target_bir_lowering=False,
        debug=False,
        enable_asserts=False,
    )
    x_d = nc.dram_tensor("x", [bpc, T, C], F32, kind="ExternalInput").ap()
    wqkv_d = nc.dram_tensor("W_qkv", [C, 3 * C], F32, kind="ExternalInput").ap()
    bqkv_d = nc.dram_tensor("b_qkv", [3 * C], F32, kind="ExternalInput").ap()
    wp_d = nc.dram_tensor("W_proj", [C, C], F32, kind="ExternalInput").ap()
    bp_d = nc.dram_tensor("b_proj", [C], F32, kind="ExternalInput").ap()
    y_d = nc.dram_tensor("y", [bpc, T, C], F32, kind="ExternalOutput").ap()

    with tile.TileContext(nc) as tc:
        with (
            tc.tile_pool(name="const", bufs=1) as cpool,
            tc.tile_pool(name="work", bufs=3) as wpool,
            tc.tile_pool(name="psum", bufs=2, space="PSUM") as ppool,
        ):
            # ---------------- global prep (once) ----------------
            # Weight loads, fp32->bf16 cast during DMA (SWDGE).
            wq = [cpool.tile([128, C], BF, tag=f"wq{i}", name=f"wq{i}") for i in range(3)]
            wk = [cpool.tile([128, C], BF, tag=f"wk{i}", name=f"wk{i}") for i in range(3)]
            wv = [cpool.tile([128, C], BF, tag=f"wv{i}", name=f"wv{i}") for i in range(3)]
            wp = [cpool.tile([128, C], BF, tag=f"wp{i}", name=f"wp{i}") for i in range(3)]
            for i in range(3):
                nc.gpsimd.dma_start(wq[i][:], wqkv_d[ts(i, 128), 0:C])
                nc.gpsimd.dma_start(wk[i][:], wqkv_d[ts(i, 128), C : 2 * C])
                nc.gpsimd.dma_start(wv[i][:], wqkv_d[ts(i, 128), 2 * C : 3 * C])
                nc.gpsimd.dma_start(wp[i][:], wp_d[ts(i, 128), :])
            # Identity for PE-mode transposes.
            ident = cpool.tile([128, 128], BF, tag="ident", name="ident")
            make_identity(nc, ident[:])
            # Transposed W_q / W_k via PE transpose (bf16, PSUM out).
            wqT = [cpool.tile([128, C], BF, tag=f"wqT{i}", name=f"wqT{i}") for i in range(3)]
            wkT = [cpool.tile([128, C], BF, tag=f"wkT{i}", name=f"wkT{i}") for i in range(3)]
            for dt_ in range(3):
                for wsrc, wdst in ((wq, wqT), (wk, wkT)):
                    ps_t = ppool.tile([128, C], BF, tag="tp", bufs=2, name="ps_t")
                    for ct in range(3):
                        nc.tensor.transpose(
                            ps_t[:, ts(ct, 128)], wsrc[ct][:, ts(dt_, 128)], ident[:]
                        )
                    nc.scalar.copy(wdst[dt_][:], ps_t[:])
            # Bias pieces.
            bqc = cpool.tile([128, 3], BF, tag="bqc", name="bqc")
            nc.gpsimd.dma_start(bqc[:], bqkv_d[0:C].rearrange("(j p) -> p j", p=128))
            bv_row = cpool.tile([1, C], BF, tag="bv_row", name="bv_row")
            nc.gpsimd.dma_start(bv_row[:], bqkv_d[2 * C : 3 * C].rearrange("(a c) -> a c", a=1))
            bp_row = cpool.tile([1, C], BF, tag="bp_row", name="bp_row")
            nc.gpsimd.dma_start(bp_row[:], bp_d.rearrange("(a c) -> a c", a=1))

            ones_row = cpool.tile([1, 128], BF, tag="ones_row", name="ones_row")
            nc.vector.memset(ones_row[:], 1.0)
            ones_col = cpool.tile([128, 1], BF, tag="ones_col", name="ones_col")
            nc.vector.memset(ones_col[:], 1.0)

            # Broadcast biases to all 128 partitions via rank-1 matmul.
            bv_rep = cpool.tile([128, C], F32, tag="bv_rep", name="bv_rep")
            ps_bv = ppool.tile([128, C], F32, tag="mm", bufs=4, name="ps_bv")
            nc.tensor.matmul(ps_bv[:], ones_row[:], bv_row[:], start=True, stop=True)
            nc.vector.tensor_copy(bv_rep[:], ps_bv[:])
            bp_rep = cpool.tile([128, C], F32, tag="bp_rep", name="bp_rep")
            ps_bp = ppool.tile([128, C], F32, tag="mm", bufs=4, name="ps_bp")
            nc.tensor.matmul(ps_bp[:], ones_row[:], bp_row[:], start=True, stop=True)
            nc.vector.tensor_copy(bp_rep[:], ps_bp[:])

            # Causal mask (allowed = upper triangular incl diag in [s, t] layout).
            tri = cpool.tile([128, 128], BF, tag="tri", name="tri")
            make_upper_triangular(nc, tri[:], val=1.0, diag=True)

            # A = W_q W_k^T / sqrt(C)   [c1, c2], bf16
            A = [cpool.tile([128, C], BF, tag=f"A{i}", name=f"A{i}") for i in range(3)]
            for c1 in range(3):
                ps_a = ppool.tile([128, C], F32, tag="mm", bufs=4, name="ps_a")
                for dt_ in range(3):
                    nc.tensor.matmul(
                        ps_a[:],
                        wqT[dt_][:, ts(c1, 128)],
                        wkT[dt_][:],
                        start=(dt_ == 0),
                        stop=(dt_ == 2),
                    )
                nc.scalar.mul(A[c1][:], ps_a[:], INV_SQRT_C)

            # u = W_k b_q / sqrt(C)  as [128, 3] bf16 (col j = c-tile j)
            u = cpool.tile([128, 3], BF, tag="u", name="u")
            for ct in range(3):
                ps_u = ppool.tile([128, 1], F32, tag="vec", bufs=2, name="ps_u")
                for dt_ in range(3):
                    nc.tensor.matmul(
                        ps_u[:],
                        wkT[dt_][:, ts(ct, 128)],
                        bqc[:, dt_ : dt_ + 1],
                        start=(dt_ == 0),
                        stop=(dt_ == 2),
                    )
                nc.scalar.mul(u[:, ct : ct + 1], ps_u[:], INV_SQRT_C)

            # ---------------- per-batch-element pipeline ----------------
            for b in range(bpc):
                # x_b cast-load [t, c] then DMA-transpose to x' [c, t]
                xb = [wpool.tile([128, C], BF, tag=f"xb{i}", name=f"xb{i}") for i in range(2)]
                for tt in range(2):
                    nc.gpsimd.dma_start(xb[tt][:], x_d[b, ts(tt, 128), :])
                xp = [wpool.tile([128, T], BF, tag=f"xp{i}", name=f"xp{i}") for i in range(3)]
                for ct in range(3):
                    ps_x = ppool.tile([128, T], BF, tag="tp", bufs=2, name="ps_x")
                    for tt in range(2):
                        nc.tensor.transpose(
                            ps_x[:, ts(tt, 128)], xb[tt][:, ts(ct, 128)], ident[:]
                        )
                    nc.scalar.copy(xp[ct][:], ps_x[:])

                # g' = (x A)^T [c2, t]
                g = [wpool.tile([128, T], BF, tag=f"g{i}", name=f"g{i}") for i in range(3)]
                for c2 in range(3):
                    ps_g = ppool.tile([128, T], F32, tag="mm", bufs=4, name="ps_g")
                    for c1 in range(3):
                        nc.tensor.matmul(
                            ps_g[:],
                            A[c1][:, ts(c2, 128)],
                            xp[c1][:],
                            start=(c1 == 0),
                            stop=(c1 == 2),
                        )
                    nc.scalar.copy(g[c2][:], ps_g[:])

                # f[s] = x_b @ u  (two per-partition columns)
                f = [wpool.tile([128, 1], F32, tag=f"f{i}", name=f"f{i}") for i in range(2)]
                for st in range(2):
                    ps_f = ppool.tile([128, 1], F32, tag="vec", bufs=2, name="ps_f")
                    for ct in range(3):
                        nc.tensor.matmul(
                            ps_f[:],
                            xp[ct][:, ts(st, 128)],
                            u[:, ct : ct + 1],
                            start=(ct == 0),
                            stop=(ct == 2),
                        )
                    nc.scalar.copy(f[st][:], ps_f[:])

                # v = x_b @ W_v + b_v   [s, c]
                v = [wpool.tile([128, C], BF, tag=f"v{i}", name=f"v{i}") for i in range(2)]
                for st in range(2):
                    ps_v = ppool.tile([128, C], F32, tag="mm", bufs=4, name="ps_v")
                    for ct in range(3):
                        nc.tensor.matmul(
                            ps_v[:],
                            xp[ct][:, ts(st, 128)],
                            wv[ct][:],
                            start=(ct == 0),
                            stop=(ct == 2),
                        )
                    nc.vector.tensor_add(v[st][:], ps_v[:], bv_rep[:])

                # S'[s, t] = x'^T g' (+ f per-partition inside exp)
                # P' = exp(S' + f) * mask
                P = [wpool.tile([128, T], BF, tag=f"P{i}", name=f"P{i}") for i in range(2)]
                ps_s0 = ppool.tile([128, T], F32, tag="mm", bufs=4, name="ps_s0")
                for ct in range(3):
                    nc.tensor.matmul(
                        ps_s0[:],
                        xp[ct][:, 0:128],
                        g[ct][:],
                        start=(ct == 0),
                        stop=(ct == 2),
                    )
                nc.scalar.activation(P[0][:], ps_s0[:], EXP, bias=f[0][:, 0:1])
                nc.vector.tensor_mul(P[0][:, 0:128], P[0][:, 0:128], tri[:])
                # s-tile 1 only attends to t in [128, 256)
                ps_s1 = ppool.tile([128, 128], F32, tag="vec", bufs=2, name="ps_s1")
                for ct in range(3):
                    nc.tensor.matmul(
                        ps_s1[:],
                        xp[ct][:, 128:256],
                        g[ct][:, 128:256],
                        start=(ct == 0),
                        stop=(ct == 2),
                    )
                nc.scalar.activation(P[1][:, 128:256], ps_s1[:], EXP, bias=f[1][:, 0:1])
                nc.vector.tensor_mul(P[1][:, 128:256], P[1][:, 128:256], tri[:])
                nc.gpsimd.memset(P[1][:, 0:128], 0.0)

                # O' = v^T P'  [c, t] (unnormalized)
                O = [wpool.tile([128, T], BF, tag=f"O{i}", name=f"O{i}") for i in range(3)]
                for ct in range(3):
                    ps_o = ppool.tile([128, T], F32, tag="mm", bufs=4, name="ps_o")
                    nc.tensor.matmul(
                        ps_o[:], v[0][:, ts(ct, 128)], P[0][:], start=True, stop=False
                    )
                    nc.tensor.matmul(
                        ps_o[:, 128:256],
                        v[1][:, ts(ct, 128)],
                        P[1][:, 128:256],
                        start=False,
                        stop=True,
                    )
                    nc.vector.tensor_copy(O[ct][:], ps_o[:])

                # softmax denominators (per t column) -> reciprocals [t, 1]
                r = [wpool.tile([128, 1], F32, tag=f"r{i}", name=f"r{i}") for i in range(2)]
                ps_d0 = ppool.tile([128, 1], F32, tag="vec", bufs=2, name="ps_d0")
                nc.tensor.matmul(ps_d0[:], P[0][:, 0:128], ones_col[:], start=True, stop=True)
                nc.vector.reciprocal(r[0][:], ps_d0[:])
                ps_d1 = ppool.tile([128, 1], F32, tag="vec", bufs=2, name="ps_d1")
                for st in range(2):
                    nc.tensor.matmul(
                        ps_d1[:],
                        P[st][:, 128:256],
                        ones_col[:],
                        start=(st == 0),
                        stop=(st == 1),
                    )
                nc.vector.reciprocal(r[1][:], ps_d1[:])

                # y = (O'^T @ W_proj) * r + b_proj   [t, d]
                for tt in range(2):
                    ps_y = ppool.tile([128, C], F32, tag="mm", bufs=4, name="ps_y")
                    for ct in range(3):
                        nc.tensor.matmul(
                            ps_y[:],
                            O[ct][:, ts(tt, 128)],
                            wp[ct][:],
                            start=(ct == 0),
                            stop=(ct == 2),
                        )
                    yo = wpool.tile([128, C], F32, tag=f"yo{tt}", name=f"yo{tt}")
                    nc.scalar.activation(
                        yo[:],
                        ps_y[:],
                        mybir.ActivationFunctionType.Copy,
                        scale=r[tt][:, 0:1],
                    )
                    nc.gpsimd.tensor_add(yo[:], yo[:], bp_rep[:])
                    nc.gpsimd.dma_start(y_d[b, ts(tt, 128), :], yo[:])

    nc.compile()
    return nc


def kernel(x, W_qkv, b_qkv, W_proj, b_proj):
    x = np.ascontiguousarray(np.asarray(x, dtype=np.float32))
    W_qkv = np.ascontiguousarray(np.asarray(W_qkv, dtype=np.float32))
    b_qkv = np.ascontiguousarray(np.asarray(b_qkv, dtype=np.float32))
    W_proj = np.ascontiguousarray(np.asarray(W_proj, dtype=np.float32))
    b_proj = np.ascontiguousarray(np.asarray(b_proj, dtype=np.float32))

    nc = build(B_PER_CORE)
    in_maps = [
        {
            "x": x[i * B_PER_CORE : (i + 1) * B_PER_CORE],
            "W_qkv": W_qkv,
            "b_qkv": b_qkv,
            "W_proj": W_proj,
            "b_proj": b_proj,
        }
        for i in range(N_CORES)
    ]
    res = run_bass_kernel_spmd(nc, in_maps, core_ids=list(range(N_CORES)))
    return np.concatenate([r["y"] for r in res.results], axis=0)


if __name__ == "__main__":
    rng = np.random.default_rng(0)
    s = 1.0 / np.sqrt(C)
    inputs = {
        "x": rng.standard_normal((B_FULL, T, C), dtype=np.float32),
        "W_qkv": rng.uniform(-s, s, (C, 3 * C)).astype(np.float32),
        "b_qkv": rng.uniform(-s, s, (3 * C,)).astype(np.float32),
        "W_proj": rng.uniform(-s, s, (C, C)).astype(np.float32),
        "b_proj": rng.uniform(-s, s, (C,)).astype(np.float32),
    }
    y = kernel(**inputs)
    print("out", y.shape, y.dtype, float(np.abs(y).max()))


# revision 9
# speedup vs baseline: 29537.4638x; 29537.4638x over previous
"""Trainium2 Bass kernel for causal self-attention (single head, scaled by full C).

Reference math (per batch element b, T=256, C=384):
    qkv = x @ W_qkv + b_qkv ;  q,k,v = split(qkv)
    S   = q @ k^T / sqrt(C) ; causal mask ; P = softmax(S)
    y   = (P @ v) @ W_proj + b_proj

Kernel restructuring (validated numerically to ~3.6e-3 rel err in bf16):
  * Fuse A = W_q W_k^T / sqrt(C) once per kernel; then
        S[t,s] = (x A x^T)[t,s] + f[s] + (terms constant over s -> dropped,
        softmax-invariant), with f = x @ (W_k b_q) / sqrt(C).
  * Work in the transposed score layout S'[s,t] so softmax statistics reduce
    over the partition dim via tiny matmuls (denominator = ones-matmul), and
    no max-subtraction is needed (|S| <= ~2, exp can't overflow).
  * All matmul operands bf16 (full-rate PE), all accumulation fp32 in PSUM.
  * Data-parallel over batch: B=512 -> 64 per core x 8 cores.
"""

import numpy as np

import concourse.bacc as bacc
import concourse.bass as bass
import concourse.mybir as mybir
import concourse.tile as tile
from concourse.bass import ts
from concourse.bass_utils import run_bass_kernel_spmd
from concourse.masks import make_identity, make_upper_triangular

T = 256
C = 384
N_CORES = 8
B_FULL = 512
B_PER_CORE = B_FULL // N_CORES  # 64

BF = mybir.dt.bfloat16
F32 = mybir.dt.float32
EXP = mybir.ActivationFunctionType.Exp
INV_SQRT_C = 1.0 / float(np.sqrt(C))


def build(bpc: int = B_PER_CORE, repeat: int = 1) -> bass.Bass:
    nc = bacc.Bacc(
        "TRN2",
        target_bir_lowering=False,
        debug=False,
        enable_asserts=False,
    )
    x_d = nc.dram_tensor("x", [bpc, T, C], F32, kind="ExternalInput").ap()
    wqkv_d = nc.dram_tensor("W_qkv", [C, 3 * C], F32, kind="ExternalInput").ap()
    bqkv_d = nc.dram_tensor("b_qkv", [3 * C], F32, kind="ExternalInput").ap()
    wp_d = nc.dram_tensor("W_proj", [C, C], F32, kind="ExternalInput").ap()
    bp_d = nc.dram_tensor("b_proj", [C], F32, kind="ExternalInput").ap()
    y_d = nc.dram_tensor("y", [bpc, T, C], F32, kind="ExternalOutput").ap()

    with tile.TileContext(nc) as tc:
        with (
            tc.tile_pool(name="const", bufs=1) as cpool,
            tc.tile_pool(name="work", bufs=3) as wpool,
            tc.tile_pool(name="psum", bufs=2, space="PSUM") as ppool,
        ):
            # ---------------- global prep (once) ----------------
            # Weight loads, fp32->bf16 cast during DMA (SWDGE).
            wq = [cpool.tile([128, C], BF, tag=f"wq{i}", name=f"wq{i}") for i in range(3)]
            wk = [cpool.tile([128, C], BF, tag=f"wk{i}", name=f"wk{i}") for i in range(3)]
            wv = [cpool.tile([128, C], BF, tag=f"wv{i}", name=f"wv{i}") for i in range(3)]
            wp = [cpool.tile([128, C], BF, tag=f"wp{i}", name=f"wp{i}") for i in range(3)]
            for i in range(3):
                nc.gpsimd.dma_start(wq[i][:], wqkv_d[ts(i, 128), 0:C])
                nc.gpsimd.dma_start(wk[i][:], wqkv_d[ts(i, 128), C : 2 * C])
                nc.gpsimd.dma_start(wv[i][:], wqkv_d[ts(i, 128), 2 * C : 3 * C])
                nc.gpsimd.dma_start(wp[i][:], wp_d[ts(i, 128), :])
            # Identity for PE-mode transposes.
            ident = cpool.tile([128, 128], BF, tag="ident", name="ident")
            make_identity(nc, ident[:])
            # Transposed W_q / W_k via PE transpose (bf16, PSUM out).
            wqT = [cpool.tile([128, C], BF, tag=f"wqT{i}", name=f"wqT{i}") for i in range(3)]
            wkT = [cpool.tile([128, C], BF, tag=f"wkT{i}", name=f"wkT{i}") for i in range(3)]
            for dt_ in range(3):
                for wsrc, wdst in ((wq, wqT), (wk, wkT)):
                    ps_t = ppool.tile([128, C], BF, tag="tp", bufs=2, name="ps_t")
                    for ct in range(3):
                        nc.tensor.transpose(
                            ps_t[:, ts(ct, 128)], wsrc[ct][:, ts(dt_, 128)], ident[:]
                        )
                    nc.scalar.copy(wdst[dt_][:], ps_t[:])
            # Bias pieces.
            bqc = cpool.tile([128, 3], BF, tag="bqc", name="bqc")
            nc.gpsimd.dma_start(bqc[:], bqkv_d[0:C].rearrange("(j p) -> p j", p=128))
            bv_row = cpool.tile([1, C], BF, tag="bv_row", name="bv_row")
            nc.gpsimd.dma_start(bv_row[:], bqkv_d[2 * C : 3 * C].rearrange("(a c) -> a c", a=1))
            bp_row = cpool.tile([1, C], BF, tag="bp_row", name="bp_row")
            nc.gpsimd.dma_start(bp_row[:], bp_d.rearrange("(a c) -> a c", a=1))

            ones_row = cpool.tile([1, 128], BF, tag="ones_row", name="ones_row")
            nc.vector.memset(ones_row[:], 1.0)
            ones_col = cpool.tile([128, 1], BF, tag="ones_col", name="ones_col")
            nc.vector.memset(ones_col[:], 1.0)

            # Broadcast biases to all 128 partitions via rank-1 matmul.
            bv_rep = cpool.tile([128, C], F32, tag="bv_rep", name="bv_rep")
            ps_bv = ppool.tile([128, C], F32, tag="mm", bufs=4, name="ps_bv")
            nc.tensor.matmul(ps_bv[:], ones_row[:], bv_row[:], start=True, stop=True)
            nc.vector.tensor_copy(bv_rep[:], ps_bv[:])
            bp_rep = cpool.tile([128, C], F32, tag="bp_rep", name="bp_rep")
            ps_bp = ppool.tile([128, C], F32, tag="mm", bufs=4, name="ps_bp")
            nc.tensor.matmul(ps_bp[:], ones_row[:], bp_row[:], start=True, stop=True)
            nc.vector.tensor_copy(bp_rep[:], ps_bp[:])

            # Causal mask (allowed = upper triangular incl diag in [s, t] layout).
            tri = cpool.tile([128, 128], BF, tag="tri", name="tri")
            make_upper_triangular(nc, tri[:], val=1.0, diag=True)

            # A = W_q W_k^T / sqrt(C)   [c1, c2], bf16
            A = [cpool.tile([128, C], BF, tag=f"A{i}", name=f"A{i}") for i in range(3)]
            for c1 in range(3):
                ps_a = ppool.tile([128, C], F32, tag="mm", bufs=4, name="ps_a")
                for dt_ in range(3):
                    nc.tensor.matmul(
                        ps_a[:],
                        wqT[dt_][:, ts(c1, 128)],
                        wkT[dt_][:],
                        start=(dt_ == 0),
                        stop=(dt_ == 2),
                    )
                nc.scalar.mul(A[c1][:], ps_a[:], INV_SQRT_C)

            # u = W_k b_q / sqrt(C)  as [128, 3] bf16 (col j = c-tile j)
            u = cpool.tile([128, 3], BF, tag="u", name="u")
            for ct in range(3):
                ps_u = ppool.tile([128, 1], F32, tag="vec", bufs=2, name="ps_u")
                for dt_ in range(3):
                    nc.tensor.matmul(
                        ps_u[:],
                        wkT[dt_][:, ts(ct, 128)],
                        bqc[:, dt_ : dt_ + 1],
                        start=(dt_ == 0),
                        stop=(dt_ == 2),
                    )
                nc.scalar.mul(u[:, ct : ct + 1], ps_u[:], INV_SQRT_C)

            # ---------------- per-batch-element pipeline ----------------
            for b in [bb for _ in range(repeat) for bb in range(bpc)]:
                # x_b cast-load [t, c] then DMA-transpose to x' [c, t]
                xb = [wpool.tile([128, C], BF, tag=f"xb{i}", name=f"xb{i}") for i in range(2)]
                for tt in range(2):
                    nc.gpsimd.dma_start(xb[tt][:], x_d[b, ts(tt, 128), :])
                xp = [wpool.tile([128, T], BF, tag=f"xp{i}", name=f"xp{i}") for i in range(3)]
                for ct in range(3):
                    ps_x = ppool.tile([128, T], BF, tag="tp", bufs=2, name="ps_x")
                    for tt in range(2):
                        nc.tensor.transpose(
                            ps_x[:, ts(tt, 128)], xb[tt][:, ts(ct, 128)], ident[:]
                        )
                    nc.scalar.copy(xp[ct][:], ps_x[:])

                # g' = (x A)^T [c2, t]
                g = [wpool.tile([128, T], BF, tag=f"g{i}", name=f"g{i}") for i in range(3)]
                for c2 in range(3):
                    ps_g = ppool.tile([128, T], F32, tag="mm", bufs=4, name="ps_g")
                    for c1 in range(3):
                        nc.tensor.matmul(
                            ps_g[:],
                            A[c1][:, ts(c2, 128)],
                            xp[c1][:],
                            start=(c1 == 0),
                            stop=(c1 == 2),
                        )
                    nc.scalar.copy(g[c2][:], ps_g[:])

                # f[s] = x_b @ u  (two per-partition columns)
                f = [wpool.tile([128, 1], F32, tag=f"f{i}", name=f"f{i}") for i in range(2)]
                for st in range(2):
                    ps_f = ppool.tile([128, 1], F32, tag="vec", bufs=2, name="ps_f")
                    for ct in range(3):
                        nc.tensor.matmul(
                            ps_f[:],
                            xp[ct][:, ts(st, 128)],
                            u[:, ct : ct + 1],
                            start=(ct == 0),
                            stop=(ct == 2),
                        )
                    nc.scalar.copy(f[st][:], ps_f[:])

                # v = x_b @ W_v + b_v   [s, c]
                v = [wpool.tile([128, C], BF, tag=f"v{i}", name=f"v{i}") for i in range(2)]
                for st in range(2):
                    ps_v = ppool.tile([128, C], F32, tag="mm", bufs=4, name="ps_v")
                    for ct in range(3):
                        nc.tensor.matmul(
                            ps_v[:],
                            xp[ct][:, ts(st, 128)],
                            wv[ct][:],
                            start=(ct == 0),
                            stop=(ct == 2),
                        )
                    nc.vector.tensor_add(v[st][:], ps_v[:], bv_rep[:])

                # S'[s, t] = x'^T g' (+ f per-partition inside exp)
                # P' = exp(S' + f) * mask
                P = [wpool.tile([128, T], BF, tag=f"P{i}", name=f"P{i}") for i in range(2)]
                ps_s0 = ppool.tile([128, T], F32, tag="mm", bufs=4, name="ps_s0")
                for ct in range(3):
                    nc.tensor.matmul(
                        ps_s0[:],
                        xp[ct][:, 0:128],
                        g[ct][:],
                        start=(ct == 0),
                        stop=(ct == 2),
                    )
                nc.scalar.activation(P[0][:], ps_s0[:], EXP, bias=f[0][:, 0:1])
                nc.vector.tensor_mul(P[0][:, 0:128], P[0][:, 0:128], tri[:])
                # s-tile 1 only attends to t in [128, 256)
                ps_s1 = ppool.tile([128, 128], F32, tag="vec", bufs=2, name="ps_s1")
                for ct in range(3):
                    nc.tensor.matmul(
                        ps_s1[:],
                        xp[ct][:, 128:256],
                        g[ct][:, 128:256],
                        start=(ct == 0),
                        stop=(ct == 2),
                    )
                nc.scalar.activation(P[1][:, 128:256], ps_s1[:], EXP, bias=f[1][:, 0:1])
                nc.vector.tensor_mul(P[1][:, 128:256], P[1][:, 128:256], tri[:])
                nc.gpsimd.memset(P[1][:, 0:128], 0.0)

                # O' = v^T P'  [c, t] (unnormalized)
                O = [wpool.tile([128, T], BF, tag=f"O{i}", name=f"O{i}") for i in range(3)]
                for ct in range(3):
                    ps_o = ppool.tile([128, T], F32, tag="mm", bufs=4, name="ps_o")
                    nc.tensor.matmul(
                        ps_o[:], v[0][:, ts(ct, 128)], P[0][:], start=True, stop=False
                    )
                    nc.tensor.matmul(
                        ps_o[:, 128:256],
                        v[1][:, ts(ct, 128)],
                        P[1][:, 128:256],
                        start=False,
                        stop=True,
                    )
                    nc.vector.tensor_copy(O[ct][:], ps_o[:])

                # softmax denominators (per t column) -> reciprocals [t, 1]
                r = [wpool.tile([128, 1], F32, tag=f"r{i}", name=f"r{i}") for i in range(2)]
                ps_d0 = ppool.tile([128, 1], F32, tag="vec", bufs=2, name="ps_d0")
                nc.tensor.matmul(ps_d0[:], P[0][:, 0:128], ones_col[:], start=True, stop=True)
                nc.vector.reciprocal(r[0][:], ps_d0[:])
                ps_d1 = ppool.tile([128, 1], F32, tag="vec", bufs=2, name="ps_d1")
                for st in range(2):
                    nc.tensor.matmul(
                        ps_d1[:],
                        P[st][:, 128:256],
                        ones_col[:],
                        start=(st == 0),
                        stop=(st == 1),
                    )
                nc.vector.reciprocal(r[1][:], ps_d1[:])

                # y = (O'^T @ W_proj) * r + b_proj   [t, d]
                for tt in range(2):
                    ps_y = ppool.tile([128, C], F32, tag="mm", bufs=4, name="ps_y")
                    for ct in range(3):
                        nc.tensor.matmul(
                            ps_y[:],
                            O[ct][:, ts(tt, 128)],
                            wp[ct][:],
                            start=(ct == 0),
                            stop=(ct == 2),
                        )
                    yo = wpool.tile([128, C], F32, tag=f"yo{tt}", name=f"yo{tt}")
                    nc.scalar.activation(
                        yo[:],
                        ps_y[:],
                        mybir.ActivationFunctionType.Copy,
                        scale=r[tt][:, 0:1],
                    )
                    nc.gpsimd.tensor_add(yo[:], yo[:], bp_rep[:])
                    nc.gpsimd.dma_start(y_d[b, ts(tt, 128), :], yo[:])

    nc.compile()
    return nc


def kernel(x, W_qkv, b_qkv, W_proj, b_proj):
    x = np.ascontiguousarray(np.asarray(x, dtype=np.float32))
    W_qkv = np.ascontiguousarray(np.asarray(W_qkv, dtype=np.float32))
    b_qkv = np.ascontiguousarray(np.asarray(b_qkv, dtype=np.float32))
    W_proj = np.ascontiguousarray(np.asarray(W_proj, dtype=np.float32))
    b_proj = np.ascontiguousarray(np.asarray(b_proj, dtype=np.float32))

    nc = build(B_PER_CORE)
    in_maps = [
        {
            "x": x[i * B_PER_CORE : (i + 1) * B_PER_CORE],
            "W_qkv": W_qkv,
            "b_qkv": b_qkv,
            "W_proj": W_proj,
            "b_proj": b_proj,
        }
        for i in range(N_CORES)
    ]
    res = run_bass_kernel_spmd(nc, in_maps, core_ids=list(range(N_CORES)))
    return np.concatenate([r["y"] for r in res.results], axis=0)


if __name__ == "__main__":
    rng = np.random.default_rng(0)
    s = 1.0 / np.sqrt(C)
    inputs = {
        "x": rng.standard_normal((B_FULL, T, C), dtype=np.float32),
        "W_qkv": rng.uniform(-s, s, (C, 3 * C)).astype(np.float32),
        "b_qkv": rng.uniform(-s, s, (3 * C,)).astype(np.float32),
        "W_proj": rng.uniform(-s, s, (C, C)).astype(np.float32),
        "b_proj": rng.uniform(-s, s, (C,)).astype(np.float32),
    }
    y = kernel(**inputs)
    print("out", y.shape, y.dtype, float(np.abs(y).max()))
